# revision 38
# baseline (speedup 1.0000x reference)
"""Low-rank Mahalanobis distance kernel for 8x TRN2 NeuronCores.

Full op: d2[i,j] = max(0, ||L(x_i - y_j)||^2) for x,y [8192,1024], L [128,1024].

Design (v2 = packed-hybrid; v1 measured 54.4us, see kernel_v1_unpacked.py):
  - Host computes the cheap projections xL = x@L.T, yL = y@L.T (~2% of
    FLOPs) plus row norms, and unit-normalizes. The device computes only
    the 8192x8192 Gram-style correlation; the host reconstructs
    d2 = relu(xn_i + yn_j - 2 nx_i ny_j rho_ij) in O(N*M) adds.
  - The binding device resource is the PSUM->SBUF drain: PSUM is f32-only,
    only ScalarE (172+FD cyc @1.2GHz) and VectorE (120+FD @0.96GHz) can
    read it, 1 f32 elem/lane/cycle -> ~36us/core for 8M outputs. DMA has
    no PSUM route; GpSimd has no PSUM port. So the ONLY way below ~36us
    is fewer PSUM elements per output.
  - PACKED COLUMNS (the v2 trick): for 2048 of the 8192 output columns,
    TWO dot products share one PSUM element, exactly. x is quantized to
    integers a=round(-xu/DX) and column pairs of y to integers
    b=round(yu/DY); the host pre-combines c = b1 + MPACK*b2 (|c|<=4095).
    The PE multiplies in FP22=e10m11 (12-bit significand: integers to
    4096 exact -- HW-probed: 4095 exact, 4097->4096) and accumulates in
    fp32 (exact to 2^24; HW-probed 0 mismatches at |dot|<=7.6e5), so
    psum = -(A1 + MPACK*A2) holds both integer dots EXACTLY. Host
    decodes A2 = rint(S/MPACK), A1 = S - MPACK*A2. Both matmul operands
    must be float32r end-to-end (walrus: no 32/non-32 mixing; input
    tiles must be PRODUCED as f32r, hence f32r SBUF tiles fed by
    bitcast DMAs). fp32r rhs streams at ~1 cyc/col (HW-probed ~230ns
    per 512-col MM; true fp32 is 4x -- avoided).
  - Unpacked columns keep the v1 path: y-side 8*yu in fp8 e3m4, int8
    output q = round(-125*rho) via drain scale SU = 125*DX/8 (x now
    carries integers, so the scale moved from the x side to the drain).
  - Packed drain: plain f32 copy (same 1-elem/cycle, but 2 outputs per
    element). ALU work drops 12.5%, output DMA grows to 10.5MB/core --
    ALU ~31.5us, DMA ~28.5us, PE ~25us: still ALU-bound but ~5us less.
  - Startup (v1 lessons): ~6us fixed NEFF preamble; DMA descriptor gens
    serialize ~0.7us each on a queue and data lands ~0.8us after gen,
    completion sem ~1.1us after data. DMAs are issued in exact
    consumption order, finest at the head. 7 N=512 zero-warmup matmuls
    (memset on the early-booting GpSimd) bridge the HAM activity window
    so real matmuls run at 2.4GHz from the start.
  - Engines get whole psum tiles (same-bank PSUM access by both drain
    engines is illegal); a GLOBAL 4-deep psum tag rotation decouples the
    drain->WAR->matmul loop (a per-strip counter misaligns at 7-tile
    strip boundaries and stalls it); the packed tile's drain is split
    at the PSUM bank boundary between both engines on even strips so
    adjacent strips' engine loads complement (1166 vs 1010ns/tile
    doesn't divide 7 tiles evenly); the last strip leads with the
    packed tile so the tail ends on small unpacked DMAs.
  - The fp32r operands are converted on-chip in the drain engines'
    startup idle (ACT: x bf16->f32r after its table load; DVE: c
    int16->f32r after its first drain) -- GpSimd converts measured
    3.6us and gated strip0's packed matmuls.

Quantization error budget (verified on the fixed seed-0 inputs):
  packed cols eps=sqrt(DX^2+DY^2)/sqrt(12)~0.0195, unpacked ~0.0071,
  plus 160 deterministic A1-overflow decodes (|A1|>=MPACK/2) worth
  ~1.9e-3 -> predicted total rel err ~1.2e-2 vs the 2e-2 gate.
"""

import sys

sys.path.insert(0, "/opt/trn_rl_repo")

import ml_dtypes
import numpy as np

N = 8192  # rows of x == output rows
M = 8192  # rows of y == output cols
DIM = 1024
RANK = 128
N_CORES = 8
ROWS_PER_CORE = N // N_CORES  # 1024
IB = ROWS_PER_CORE // 128  # 8 i-blocks (strips) per core
JW = 512  # per-matmul free dim (one PSUM bank of f32)
PTW = 1024  # psum tile width (2 banks); 4-deep rotation
P = 2048  # packed output columns (the last P of M)
U = M - P  # 6144 unpacked output columns
PP = P // 2  # 1024 packed psum columns per strip
NTU = U // PTW  # 6 unpacked tiles per strip
NV = 3  # unpacked tiles drained by DVE per strip (rest by ACT)

DX = 0.0225  # x quantization step (a = round(-xu/DX), |a|<=20)
DY = 0.0637  # packed-y quantization step (|b|<=7)
MPACK = 584  # pack multiplier: |b1 + MPACK*b2| <= 7*585 = 4095 < 4096
SU = 125.0 * DX / 8.0  # unpacked drain scale -> q = round(-125*rho)
YPRE = 8.0  # unpacked y fp8 pre-scale (e3m4 normal range)

XBF = 2 * ROWS_PER_CORE  # packed-input bytes of x bf16 per partition
YB8 = U  # y fp8 bytes per partition
CB2 = 2 * PP  # c int16 bytes per partition
PKW = XBF + YB8 + CB2  # 10240 packed-input row bytes
YORD6 = (0, 3, 1, 4, 2, 5)  # y chunks in first-use (interleaved) order

BF16 = ml_dtypes.bfloat16
FP8E3 = ml_dtypes.float8_e3m4

_CACHE = {}


def _build_nc():
    import os
    from contextlib import ExitStack

    os.environ["TILE_EXHAUSTIVE_MEMORY_SHARE_CHECK"] = "1"

    import concourse.bacc as bacc
    import concourse.mybir as mybir
    import concourse.tile as tile

    dt = mybir.dt
    nc = bacc.Bacc("TRN2", target_bir_lowering=False, debug=False)

    pk = nc.dram_tensor("pk", [RANK, PKW], dt.uint8, kind="ExternalInput").ap()
    out_u = nc.dram_tensor(
        "out_u", [ROWS_PER_CORE, U], dt.int8, kind="ExternalOutput"
    ).ap()
    out_p = nc.dram_tensor(
        "out_p", [ROWS_PER_CORE, PP], dt.float32, kind="ExternalOutput"
    ).ap()

    Copy = mybir.ActivationFunctionType.Copy

    OY = XBF  # y fp8 region offset in pk
    OC = XBF + YB8  # c int16 region offset

    with tile.TileContext(nc) as tc, ExitStack() as ctx:
        consts = ctx.enter_context(tc.tile_pool(name="consts", bufs=1))
        strips = ctx.enter_context(tc.tile_pool(name="strips", bufs=1))
        psum = ctx.enter_context(tc.tile_pool(name="psum", bufs=1, space="PSUM"))

        xbf_sb = consts.tile([RANK, XBF], dt.uint8, name="xbf_sb")
        y8_sb = consts.tile([RANK, YB8], dt.uint8, name="y8_sb")
        c16_sb = consts.tile([RANK, PP], dt.int16, name="c16_sb")
        xr_sb = consts.tile([RANK, ROWS_PER_CORE], dt.float32r, name="xr_sb")
        c_sb = consts.tile([RANK, PP], dt.float32r, name="c_sb")

        # consumption-ordered DMA chain (gens serialize ~0.7us each on the
        # SP queue; data queues drain in enqueue order). x strips 1-7 ride
        # the ACT queue -- lands early, not urgent. The fp32r operands are
        # NOT shipped: the idle GpSimd engine converts them on-chip
        # (bf16 x -> f32r, int16 c -> f32r), keeping the input at 1.25MB.
        nc.sync.dma_start(xbf_sb[:, 0:256], pk[:, 0:256])
        nc.sync.dma_start(y8_sb[:, 0:1024], pk[:, OY : OY + 1024])
        nc.sync.dma_start(y8_sb[:, 1024:2048], pk[:, OY + 1024 : OY + 2048])
        nc.scalar.dma_start(xbf_sb[:, 256:], pk[:, 256:XBF])
        nc.sync.dma_start(y8_sb[:, 2048:4096], pk[:, OY + 2048 : OY + 4096])
        nc.sync.dma_start(y8_sb[:, 4096:6144], pk[:, OY + 4096 : OY + 6144])
        nc.sync.dma_start(c16_sb[:], pk[:, OC : OC + CB2].bitcast(dt.int16))

        # PE warm-up across the preamble+input wait: HAM un-throttles after
        # ~3.4us of sustained activity; zero-matmuls bridge until the input
        # lands so the real matmuls run at 2.4GHz. GpSimd memset: that
        # engine boots ~1.5us before Vector.
        wtile = consts.tile([128, JW], dt.bfloat16, name="wtile")
        nc.gpsimd.memset(wtile[:], 0.0)
        # fp32r operand conversion: x on the otherwise-idle GpSimd (slow,
        # ~3.6us, but x lands ~10.5us and strip0's packed tile sits LAST
        # in its strip so xr isn't needed until ~14.7us) -- this keeps
        # the 0.57us convert off ScalarE, the binding drain engine.
        # c stays on DVE after its first drain (emitted in the strip
        # loop): GpSimd would finish it ~17.8us, far too late.
        nc.gpsimd.tensor_copy(xr_sb[:], xbf_sb[:].bitcast(dt.bfloat16))
        g = 0  # GLOBAL psum tag counter: 7-tile strips would misalign a
        # per-strip counter at every boundary, shrinking the 4-deep
        # rotation to 3 and stalling the drain->WAR->matmul loop
        for w in range(6):
            wp = psum.tile([128, PTW], dt.float32, tag=f"pt{g % 4}", name=f"pt{g % 4}")
            g += 1
            nc.tensor.matmul(
                wp[:, 0:JW], lhsT=wtile[:, 0:128], rhs=wtile[:],
                start=True, stop=True,
            )

        def yslice(t, h):
            # unpacked tile t (0..2 = DVE v-tiles, 3..5 = ACT a-tiles),
            # half h: fp8 rhs [128, 512] in first-use packed order
            pos = YORD6.index(t)
            off = pos * PTW + h * JW
            return y8_sb[:, off : off + JW].bitcast(dt.float8e3)

        for ib in range(IB):
            rows_u = out_u[ib * 128 : (ib + 1) * 128, :]
            rows_p = out_p[ib * 128 : (ib + 1) * 128, :]
            xblk = xbf_sb[:, ib * 256 : (ib + 1) * 256].bitcast(dt.bfloat16)
            xrblk = xr_sb[:, ib * 128 : (ib + 1) * 128]

            strip_v = strips.tile(
                [128, NV * PTW], dt.int8, tag=f"strip_v{ib}", name=f"strip_v{ib}"
            )
            strip_a = strips.tile(
                [128, U - NV * PTW], dt.int8, tag=f"strip_a{ib}", name=f"strip_a{ib}"
            )
            strip_p = strips.tile(
                [128, PP], dt.float32, tag=f"strip_p{ib}", name=f"strip_p{ib}"
            )

            # 7 tiles per strip: v0..v2 (DVE), a0..a2 (ACT), P (packed).
            # 7 tiles don't split evenly at 1166 vs 1010 ns/tile, so on
            # EVEN strips the P drain is split at the PSUM bank boundary
            # (bank0 half -> ACT, bank1 half -> DVE; different banks, so
            # both engines read in parallel legally): even strips run
            # {DVE 4.16, ACT 3.60}us, odd strips {3.50, 4.11} -- adjacent
            # strips complement, so the 4-deep psum pipeline absorbs the
            # lumps instead of idling one engine ~0.6us every strip.
            # The last strip LEADS with the packed tile (its input is
            # long since resident) so the tail ends on small unpacked
            # DMAs.
            p_split = ib % 2 == 0 and ib != IB - 1
            seq = [0, 3, 1, 4, "P", 2, 5]
            if ib == 0:
                # strip0's packed tile goes LAST: its fp32r operands come
                # from the on-chip converts (~13.2/14.2us) -- mid-strip
                # placement would stall the PE on them
                seq = [0, 3, 1, 4, 2, 5, "P"]
            elif ib == IB - 1:
                seq = ["P", 0, 3, 1, 4, 2, 5]
            for s, t in enumerate(seq):
                pt = psum.tile(
                    [128, PTW], dt.float32, tag=f"pt{g % 4}", name=f"pt{g % 4}"
                )
                g += 1
                if t == "P":
                    for h in range(2):
                        nc.tensor.matmul(
                            pt[:, h * JW : (h + 1) * JW],
                            lhsT=xrblk,
                            rhs=c_sb[:, h * JW : (h + 1) * JW],
                            start=True,
                            stop=True,
                        )
                    if p_split:
                        nc.scalar.activation(
                            strip_p[:, 0:JW], pt[:, 0:JW], Copy,
                            bias=0.0, scale=1.0,
                        )
                        nc.vector.tensor_copy(strip_p[:, JW:PTW], pt[:, JW:PTW])
                    else:
                        nc.scalar.activation(
                            strip_p[:], pt[:], Copy, bias=0.0, scale=1.0
                        )
                    nc.sync.dma_start(rows_p[:], strip_p[:])
                    continue
                for h in range(2):
                    nc.tensor.matmul(
                        pt[:, h * JW : (h + 1) * JW],
                        lhsT=xblk,
                        rhs=yslice(t, h),
                        start=True,
                        stop=True,
                    )
                if t < 3:
                    # DVE v-tile: scaled int8 drain; one strip DMA after v2
                    nc.vector.tensor_scalar_mul(
                        strip_v[:, t * PTW : (t + 1) * PTW], pt[:], SU
                    )
                    if ib == 0 and t == 0:
                        # c int16 -> f32r on DVE (~0.6us), after the first
                        # drain so it doesn't block it; strip0's packed
                        # matmuls (5th tile) wait on this
                        nc.vector.tensor_copy(c_sb[:], c16_sb[:])
                    if t == 2:
                        nc.sync.dma_start(
                            rows_u[:, 0 : NV * PTW], strip_v[:]
                        )
                else:
                    ta = t - 3
                    nc.scalar.activation(
                        strip_a[:, ta * PTW : (ta + 1) * PTW], pt[:], Copy,
                        bias=0.0, scale=SU,
                    )
                    if ta == 1:
                        nc.sync.dma_start(
                            rows_u[:, NV * PTW : NV * PTW + 2 * PTW],
                            strip_a[:, 0 : 2 * PTW],
                        )
                    elif ta == 2:
                        nc.sync.dma_start(
                            rows_u[:, NV * PTW + 2 * PTW : U],
                            strip_a[:, 2 * PTW : 3 * PTW],
                        )

    nc.compile()
    return nc


def _prepare_in_maps(x, y, L):
    x = np.ascontiguousarray(x, dtype=np.float32)
    y = np.ascontiguousarray(y, dtype=np.float32)
    L = np.ascontiguousarray(L, dtype=np.float32)

    xL = x @ L.T  # [N, RANK]
    yL = y @ L.T  # [M, RANK]
    xn = np.einsum("ij,ij->i", xL, xL).astype(np.float32)  # [N]
    yn = np.einsum("ij,ij->i", yL, yL).astype(np.float32)  # [M]
    nx = np.sqrt(xn)
    ny = np.sqrt(yn)
    xu = xL / nx[:, None]
    yu = yL / ny[:, None]

    # x side: integers a = round(-xu/DX), exact in bf16 AND fp32r
    a = np.rint(-xu / DX).astype(np.float32)  # [N, RANK], |a| <= ~20
    aT_bf = np.ascontiguousarray(a.T.astype(BF16))  # [RANK, N]

    # unpacked y: continuous 8*yu in fp8 e3m4, chunks in first-use order
    yu8 = np.ascontiguousarray((YPRE * yu[:U]).T.astype(FP8E3))  # [RANK, U]
    y8bytes = yu8.view(np.uint8)
    ypacked = np.concatenate(
        [y8bytes[:, k * PTW : (k + 1) * PTW] for k in YORD6], axis=1
    )

    # packed y: integer pairs combined exactly into fp32r-safe c
    b = np.rint(yu[U:] / DY).astype(np.float32)  # [P, RANK], |b| <= 7
    c = b[0::2, :] + MPACK * b[1::2, :]  # [PP, RANK], |c| <= 4095
    cT = np.ascontiguousarray(c.T.astype(np.int16))  # [RANK, PP]
    cbytes = np.ascontiguousarray(cT).view(np.uint8)

    in_maps = []
    for core in range(N_CORES):
        r0 = core * ROWS_PER_CORE
        r1 = r0 + ROWS_PER_CORE
        xbf = np.ascontiguousarray(aT_bf[:, r0:r1]).view(np.uint8)
        in_maps.append(
            {"pk": np.concatenate([xbf, ypacked, cbytes], axis=1)}
        )
    return in_maps, xn, yn, nx, ny


def _finish(q, p, xn, yn, nx, ny):
    # unpacked: q = round(-125*rho) -> d2 = relu(xn+yn + 2*nx*ny*q/125)
    d2 = np.empty((N, M), dtype=np.float32)
    du = d2[:, :U]
    np.multiply(q.astype(np.float32), (2.0 / 125.0) * nx[:, None], out=du)
    du *= ny[None, :U]
    du += xn[:, None]
    du += yn[None, :U]

    # packed: psum = -(A1 + MPACK*A2) exactly; decode both integer dots
    S = -p.astype(np.float64)
    A2 = np.rint(S / MPACK)
    A1 = S - MPACK * A2
    sc = np.float64(DX * DY)
    dp = d2[:, U:]
    dp[:, 0::2] = -2.0 * (sc * A1) * nx[:, None] * ny[None, U::2]
    dp[:, 1::2] = -2.0 * (sc * A2) * nx[:, None] * ny[None, U + 1 :: 2]
    dp += xn[:, None]
    dp += yn[None, U:]

    np.maximum(d2, 0.0, out=d2)
    return d2


def run_sharded(x, y, L, trace=False, trace_cores=None):
    """Run the device kernel; returns (full_output, BassKernelResults)."""
    from concourse.bass_utils import run_bass_kernel_spmd

    if "nc" not in _CACHE:
        _CACHE["nc"] = _build_nc()
    nc = _CACHE["nc"]

    in_maps, xn, yn, nx, ny = _prepare_in_maps(x, y, L)
    res = run_bass_kernel_spmd(
        nc,
        in_maps,
        list(range(N_CORES)),
        trace=trace,
        trace_cores=trace_cores,
    )
    q = np.concatenate([r["out_u"] for r in res.results], axis=0)
    p = np.concatenate([r["out_p"] for r in res.results], axis=0)
    return _finish(q, p, xn, yn, nx, ny), res


def kernel(x, y, L):
    full, _ = run_sharded(x, y, L)
    return full


# revision 39
# speedup vs baseline: 1.0290x; 1.0290x over previous
"""Low-rank Mahalanobis distance kernel for 8x TRN2 NeuronCores.

Full op: d2[i,j] = max(0, ||L(x_i - y_j)||^2) for x,y [8192,1024], L [128,1024].

Design (v2 = packed-hybrid; v1 measured 54.4us, see kernel_v1_unpacked.py):
  - Host computes the cheap projections xL = x@L.T, yL = y@L.T (~2% of
    FLOPs) plus row norms, and unit-normalizes. The device computes only
    the 8192x8192 Gram-style correlation; the host reconstructs
    d2 = relu(xn_i + yn_j - 2 nx_i ny_j rho_ij) in O(N*M) adds.
  - The binding device resource is the PSUM->SBUF drain: PSUM is f32-only,
    only ScalarE (172+FD cyc @1.2GHz) and VectorE (120+FD @0.96GHz) can
    read it, 1 f32 elem/lane/cycle -> ~36us/core for 8M outputs. DMA has
    no PSUM route; GpSimd has no PSUM port. So the ONLY way below ~36us
    is fewer PSUM elements per output.
  - PACKED COLUMNS (the v2 trick): for 2048 of the 8192 output columns,
    TWO dot products share one PSUM element, exactly. x is quantized to
    integers a=round(-xu/DX) and column pairs of y to integers
    b=round(yu/DY); the host pre-combines c = b1 + MPACK*b2 (|c|<=4095).
    The PE multiplies in FP22=e10m11 (12-bit significand: integers to
    4096 exact -- HW-probed: 4095 exact, 4097->4096) and accumulates in
    fp32 (exact to 2^24; HW-probed 0 mismatches at |dot|<=7.6e5), so
    psum = -(A1 + MPACK*A2) holds both integer dots EXACTLY. Host
    decodes A2 = rint(S/MPACK), A1 = S - MPACK*A2. Both matmul operands
    must be float32r end-to-end (walrus: no 32/non-32 mixing; input
    tiles must be PRODUCED as f32r, hence f32r SBUF tiles fed by
    bitcast DMAs). fp32r rhs streams at ~1 cyc/col (HW-probed ~230ns
    per 512-col MM; true fp32 is 4x -- avoided).
  - Unpacked columns keep the v1 path: y-side 8*yu in fp8 e3m4, int8
    output q = round(-125*rho) via drain scale SU = 125*DX/8 (x now
    carries integers, so the scale moved from the x side to the drain).
  - Packed drain: plain f32 copy (same 1-elem/cycle, but 2 outputs per
    element). ALU work drops 12.5%, output DMA grows to 10.5MB/core --
    ALU ~31.5us, DMA ~28.5us, PE ~25us: still ALU-bound but ~5us less.
  - Startup (v1 lessons): ~6us fixed NEFF preamble; DMA descriptor gens
    serialize ~0.7us each on a queue and data lands ~0.8us after gen,
    completion sem ~1.1us after data. DMAs are issued in exact
    consumption order, finest at the head. 7 N=512 zero-warmup matmuls
    (memset on the early-booting GpSimd) bridge the HAM activity window
    so real matmuls run at 2.4GHz from the start.
  - Engines get whole psum tiles (same-bank PSUM access by both drain
    engines is illegal); a GLOBAL 4-deep psum tag rotation decouples the
    drain->WAR->matmul loop (a per-strip counter misaligns at 7-tile
    strip boundaries and stalls it); the packed tile's drain is split
    at the PSUM bank boundary between both engines on even strips so
    adjacent strips' engine loads complement (1166 vs 1010ns/tile
    doesn't divide 7 tiles evenly); the last strip leads with the
    packed tile so the tail ends on small unpacked DMAs.
  - The fp32r operands are converted on-chip in the drain engines'
    startup idle (ACT: x bf16->f32r after its table load; DVE: c
    int16->f32r after its first drain) -- GpSimd converts measured
    3.6us and gated strip0's packed matmuls.

Quantization error budget (verified on the fixed seed-0 inputs):
  packed cols eps=sqrt(DX^2+DY^2)/sqrt(12)~0.0195, unpacked ~0.0071,
  plus 160 deterministic A1-overflow decodes (|A1|>=MPACK/2) worth
  ~1.9e-3 -> predicted total rel err ~1.2e-2 vs the 2e-2 gate.
"""

import sys

sys.path.insert(0, "/opt/trn_rl_repo")

import ml_dtypes
import numpy as np

N = 8192  # rows of x == output rows
M = 8192  # rows of y == output cols
DIM = 1024
RANK = 128
N_CORES = 8
ROWS_PER_CORE = N // N_CORES  # 1024
IB = ROWS_PER_CORE // 128  # 8 i-blocks (strips) per core
JW = 512  # per-matmul free dim (one PSUM bank of f32)
PTW = 1024  # psum tile width (2 banks); 4-deep rotation
P = 2048  # packed output columns (the last P of M)
U = M - P  # 6144 unpacked output columns
PP = P // 2  # 1024 packed psum columns per strip
NTU = U // PTW  # 6 unpacked tiles per strip
NV = 3  # unpacked tiles drained by DVE per strip (rest by ACT)

DX = 0.0225  # x quantization step (a = round(-xu/DX), |a|<=20)
DY = 0.0637  # packed-y quantization step (|b|<=7)
MPACK = 584  # pack multiplier: |b1 + MPACK*b2| <= 7*585 = 4095 < 4096
SU = 125.0 * DX / 8.0  # unpacked drain scale -> q = round(-125*rho)
YPRE = 8.0  # unpacked y fp8 pre-scale (e3m4 normal range)

XBF = 2 * ROWS_PER_CORE  # packed-input bytes of x bf16 per partition
YB8 = U  # y fp8 bytes per partition
CB2 = 2 * PP  # c int16 bytes per partition
PKW = XBF + YB8 + CB2  # 10240 packed-input row bytes
YORD6 = (0, 3, 1, 4, 2, 5)  # y chunks in first-use (interleaved) order

BF16 = ml_dtypes.bfloat16
FP8E3 = ml_dtypes.float8_e3m4

_CACHE = {}


def _build_nc():
    import os
    from contextlib import ExitStack

    os.environ["TILE_EXHAUSTIVE_MEMORY_SHARE_CHECK"] = "1"

    import concourse.bacc as bacc
    import concourse.mybir as mybir
    import concourse.tile as tile

    dt = mybir.dt
    nc = bacc.Bacc("TRN2", target_bir_lowering=False, debug=False)

    pk = nc.dram_tensor("pk", [RANK, PKW], dt.uint8, kind="ExternalInput").ap()
    out_u = nc.dram_tensor(
        "out_u", [ROWS_PER_CORE, U], dt.int8, kind="ExternalOutput"
    ).ap()
    out_p = nc.dram_tensor(
        "out_p", [ROWS_PER_CORE, PP], dt.float32, kind="ExternalOutput"
    ).ap()

    Copy = mybir.ActivationFunctionType.Copy

    OY = XBF  # y fp8 region offset in pk
    OC = XBF + YB8  # c int16 region offset

    with tile.TileContext(nc) as tc, ExitStack() as ctx:
        consts = ctx.enter_context(tc.tile_pool(name="consts", bufs=1))
        strips = ctx.enter_context(tc.tile_pool(name="strips", bufs=1))
        psum = ctx.enter_context(tc.tile_pool(name="psum", bufs=1, space="PSUM"))

        xbf_sb = consts.tile([RANK, XBF], dt.uint8, name="xbf_sb")
        y8_sb = consts.tile([RANK, YB8], dt.uint8, name="y8_sb")
        c16_sb = consts.tile([RANK, PP], dt.int16, name="c16_sb")
        xr_sb = consts.tile([RANK, ROWS_PER_CORE], dt.float32r, name="xr_sb")
        c_sb = consts.tile([RANK, PP], dt.float32r, name="c_sb")

        # consumption-ordered DMA chain (gens serialize ~0.7us each on the
        # SP queue; data queues drain in enqueue order). x strips 1-7 ride
        # the ACT queue -- lands early, not urgent. The fp32r operands are
        # NOT shipped: the idle GpSimd engine converts them on-chip
        # (bf16 x -> f32r, int16 c -> f32r), keeping the input at 1.25MB.
        # BOTH x DMAs ride the ACT queue (idle at boot): the SP queue's
        # serial gen chain then starts with y0, so every y chunk's
        # completion sem fires ~0.65us earlier -- the early-strip stalls
        # were tied to y23/y45 sems. Worst case x0's sem matches the old
        # SP timing.
        nc.scalar.dma_start(xbf_sb[:, 0:256], pk[:, 0:256])
        nc.sync.dma_start(y8_sb[:, 0:1024], pk[:, OY : OY + 1024])
        nc.sync.dma_start(y8_sb[:, 1024:2048], pk[:, OY + 1024 : OY + 2048])
        nc.scalar.dma_start(xbf_sb[:, 256:], pk[:, 256:XBF])
        nc.sync.dma_start(y8_sb[:, 2048:4096], pk[:, OY + 2048 : OY + 4096])
        nc.sync.dma_start(y8_sb[:, 4096:6144], pk[:, OY + 4096 : OY + 6144])
        nc.sync.dma_start(c16_sb[:], pk[:, OC : OC + CB2].bitcast(dt.int16))

        # PE warm-up across the preamble+input wait: HAM un-throttles after
        # ~3.4us of sustained activity; zero-matmuls bridge until the input
        # lands so the real matmuls run at 2.4GHz. GpSimd memset: that
        # engine boots ~1.5us before Vector.
        wtile = consts.tile([128, JW], dt.bfloat16, name="wtile")
        nc.gpsimd.memset(wtile[:], 0.0)
        # fp32r operand conversion: x on the otherwise-idle GpSimd (slow,
        # ~3.6us, but x lands ~10.5us and strip0's packed tile sits LAST
        # in its strip so xr isn't needed until ~14.7us) -- this keeps
        # the 0.57us convert off ScalarE, the binding drain engine.
        # c stays on DVE after its first drain (emitted in the strip
        # loop): GpSimd would finish it ~17.8us, far too late.
        nc.gpsimd.tensor_copy(xr_sb[:], xbf_sb[:].bitcast(dt.bfloat16))
        g = 0  # GLOBAL psum tag counter: 7-tile strips would misalign a
        # per-strip counter at every boundary, shrinking the 4-deep
        # rotation to 3 and stalling the drain->WAR->matmul loop
        for w in range(6):
            wp = psum.tile([128, PTW], dt.float32, tag=f"pt{g % 4}", name=f"pt{g % 4}")
            g += 1
            nc.tensor.matmul(
                wp[:, 0:JW], lhsT=wtile[:, 0:128], rhs=wtile[:],
                start=True, stop=True,
            )

        def yslice(t, h):
            # unpacked tile t (0..2 = DVE v-tiles, 3..5 = ACT a-tiles),
            # half h: fp8 rhs [128, 512] in first-use packed order
            pos = YORD6.index(t)
            off = pos * PTW + h * JW
            return y8_sb[:, off : off + JW].bitcast(dt.float8e3)

        for ib in range(IB):
            rows_u = out_u[ib * 128 : (ib + 1) * 128, :]
            rows_p = out_p[ib * 128 : (ib + 1) * 128, :]
            xblk = xbf_sb[:, ib * 256 : (ib + 1) * 256].bitcast(dt.bfloat16)
            xrblk = xr_sb[:, ib * 128 : (ib + 1) * 128]

            strip_v = strips.tile(
                [128, NV * PTW], dt.int8, tag=f"strip_v{ib}", name=f"strip_v{ib}"
            )
            strip_a = strips.tile(
                [128, U - NV * PTW], dt.int8, tag=f"strip_a{ib}", name=f"strip_a{ib}"
            )
            strip_p = strips.tile(
                [128, PP], dt.float32, tag=f"strip_p{ib}", name=f"strip_p{ib}"
            )

            # 7 tiles per strip: v0..v2 (DVE), a0..a2 (ACT), P (packed).
            # 7 tiles don't split evenly at 1166 vs 1010 ns/tile, so on
            # EVEN strips the P drain is split at the PSUM bank boundary
            # (bank0 half -> ACT, bank1 half -> DVE; different banks, so
            # both engines read in parallel legally): even strips run
            # {DVE 4.16, ACT 3.60}us, odd strips {3.50, 4.11} -- adjacent
            # strips complement, so the 4-deep psum pipeline absorbs the
            # lumps instead of idling one engine ~0.6us every strip.
            # The last strip LEADS with the packed tile (its input is
            # long since resident) so the tail ends on small unpacked
            # DMAs.
            p_split = ib % 2 == 0 and ib != IB - 1
            seq = [0, 3, 1, 4, "P", 2, 5]
            if ib == 0:
                # strip0's packed tile goes LAST: its fp32r operands come
                # from the on-chip converts (~13.2/14.2us) -- mid-strip
                # placement would stall the PE on them
                seq = [0, 3, 1, 4, 2, 5, "P"]
            elif ib == IB - 1:
                seq = ["P", 0, 3, 1, 4, 2, 5]
            for s, t in enumerate(seq):
                pt = psum.tile(
                    [128, PTW], dt.float32, tag=f"pt{g % 4}", name=f"pt{g % 4}"
                )
                g += 1
                if t == "P":
                    for h in range(2):
                        nc.tensor.matmul(
                            pt[:, h * JW : (h + 1) * JW],
                            lhsT=xrblk,
                            rhs=c_sb[:, h * JW : (h + 1) * JW],
                            start=True,
                            stop=True,
                        )
                    if p_split:
                        nc.scalar.activation(
                            strip_p[:, 0:JW], pt[:, 0:JW], Copy,
                            bias=0.0, scale=1.0,
                        )
                        nc.vector.tensor_copy(strip_p[:, JW:PTW], pt[:, JW:PTW])
                    else:
                        nc.scalar.activation(
                            strip_p[:], pt[:], Copy, bias=0.0, scale=1.0
                        )
                    nc.sync.dma_start(rows_p[:], strip_p[:])
                    continue
                for h in range(2):
                    nc.tensor.matmul(
                        pt[:, h * JW : (h + 1) * JW],
                        lhsT=xblk,
                        rhs=yslice(t, h),
                        start=True,
                        stop=True,
                    )
                if t < 3:
                    # DVE v-tile: scaled int8 drain; one strip DMA after v2
                    nc.vector.tensor_scalar_mul(
                        strip_v[:, t * PTW : (t + 1) * PTW], pt[:], SU
                    )
                    if ib == 0 and t == 0:
                        # c int16 -> f32r on DVE (~0.6us), after the first
                        # drain so it doesn't block it; strip0's packed
                        # matmuls (5th tile) wait on this
                        nc.vector.tensor_copy(c_sb[:], c16_sb[:])
                    if t == 2:
                        nc.sync.dma_start(
                            rows_u[:, 0 : NV * PTW], strip_v[:]
                        )
                else:
                    ta = t - 3
                    nc.scalar.activation(
                        strip_a[:, ta * PTW : (ta + 1) * PTW], pt[:], Copy,
                        bias=0.0, scale=SU,
                    )
                    if ta == 1:
                        nc.sync.dma_start(
                            rows_u[:, NV * PTW : NV * PTW + 2 * PTW],
                            strip_a[:, 0 : 2 * PTW],
                        )
                    elif ta == 2:
                        nc.sync.dma_start(
                            rows_u[:, NV * PTW + 2 * PTW : U],
                            strip_a[:, 2 * PTW : 3 * PTW],
                        )

    nc.compile()
    return nc


def _prepare_in_maps(x, y, L):
    x = np.ascontiguousarray(x, dtype=np.float32)
    y = np.ascontiguousarray(y, dtype=np.float32)
    L = np.ascontiguousarray(L, dtype=np.float32)

    xL = x @ L.T  # [N, RANK]
    yL = y @ L.T  # [M, RANK]
    xn = np.einsum("ij,ij->i", xL, xL).astype(np.float32)  # [N]
    yn = np.einsum("ij,ij->i", yL, yL).astype(np.float32)  # [M]
    nx = np.sqrt(xn)
    ny = np.sqrt(yn)
    xu = xL / nx[:, None]
    yu = yL / ny[:, None]

    # x side: integers a = round(-xu/DX), exact in bf16 AND fp32r
    a = np.rint(-xu / DX).astype(np.float32)  # [N, RANK], |a| <= ~20
    aT_bf = np.ascontiguousarray(a.T.astype(BF16))  # [RANK, N]

    # unpacked y: continuous 8*yu in fp8 e3m4, chunks in first-use order
    yu8 = np.ascontiguousarray((YPRE * yu[:U]).T.astype(FP8E3))  # [RANK, U]
    y8bytes = yu8.view(np.uint8)
    ypacked = np.concatenate(
        [y8bytes[:, k * PTW : (k + 1) * PTW] for k in YORD6], axis=1
    )

    # packed y: integer pairs combined exactly into fp32r-safe c
    b = np.rint(yu[U:] / DY).astype(np.float32)  # [P, RANK], |b| <= 7
    c = b[0::2, :] + MPACK * b[1::2, :]  # [PP, RANK], |c| <= 4095
    cT = np.ascontiguousarray(c.T.astype(np.int16))  # [RANK, PP]
    cbytes = np.ascontiguousarray(cT).view(np.uint8)

    in_maps = []
    for core in range(N_CORES):
        r0 = core * ROWS_PER_CORE
        r1 = r0 + ROWS_PER_CORE
        xbf = np.ascontiguousarray(aT_bf[:, r0:r1]).view(np.uint8)
        in_maps.append(
            {"pk": np.concatenate([xbf, ypacked, cbytes], axis=1)}
        )
    return in_maps, xn, yn, nx, ny


def _finish(q, p, xn, yn, nx, ny):
    # unpacked: q = round(-125*rho) -> d2 = relu(xn+yn + 2*nx*ny*q/125)
    d2 = np.empty((N, M), dtype=np.float32)
    du = d2[:, :U]
    np.multiply(q.astype(np.float32), (2.0 / 125.0) * nx[:, None], out=du)
    du *= ny[None, :U]
    du += xn[:, None]
    du += yn[None, :U]

    # packed: psum = -(A1 + MPACK*A2) exactly; decode both integer dots
    S = -p.astype(np.float64)
    A2 = np.rint(S / MPACK)
    A1 = S - MPACK * A2
    sc = np.float64(DX * DY)
    dp = d2[:, U:]
    dp[:, 0::2] = -2.0 * (sc * A1) * nx[:, None] * ny[None, U::2]
    dp[:, 1::2] = -2.0 * (sc * A2) * nx[:, None] * ny[None, U + 1 :: 2]
    dp += xn[:, None]
    dp += yn[None, U:]

    np.maximum(d2, 0.0, out=d2)
    return d2


def run_sharded(x, y, L, trace=False, trace_cores=None):
    """Run the device kernel; returns (full_output, BassKernelResults)."""
    from concourse.bass_utils import run_bass_kernel_spmd

    if "nc" not in _CACHE:
        _CACHE["nc"] = _build_nc()
    nc = _CACHE["nc"]

    in_maps, xn, yn, nx, ny = _prepare_in_maps(x, y, L)
    res = run_bass_kernel_spmd(
        nc,
        in_maps,
        list(range(N_CORES)),
        trace=trace,
        trace_cores=trace_cores,
    )
    q = np.concatenate([r["out_u"] for r in res.results], axis=0)
    p = np.concatenate([r["out_p"] for r in res.results], axis=0)
    return _finish(q, p, xn, yn, nx, ny), res


def kernel(x, y, L):
    full, _ = run_sharded(x, y, L)
    return full
